# revision 1
# baseline (speedup 1.0000x reference)
"""CARAFE D2: tensor-engine banded-matmul design.

out[c, y, x] = sum_di sum_dj fpad[c, y//2+di, x//2+dj] * m[di*5+dj, y, x]

For a fixed input row index i (covering output rows y=2i and 2i+1, which use
the same feature rows) and tap row di, the contribution over all (yp, x) is a
matmul contracting over the padded input column j' (128 lanes):

    out_i[c, (yp, x)] += sum_{j'} ftT[j', r=i+di, c] * B_di[j', (yp, x)]

where B_di[j', yp, x] = m[(di, dj), 2i+yp, x] at dj = j' - x//2 + 2 (banded,
5 diagonals per yp, zeros elsewhere).  The 5 di-taps accumulate in a full
512-wide PSUM bank.  B tiles are built per i by GPSIMD local_scatter from a
host-pregathered dense tensor maskD with a static index table (fp32 values
scattered as uint16 pairs; the banded slot positions are y-independent).

The local_scatter extended-ISA instruction cannot carry semaphore ops through
this walrus build, so its sync is relocated onto adjacent Pool-engine memsets
(sound: Q7 execution is strict FIFO per engine), and a final pass splits any
instruction with more than one wait into standalone sequencer NOPs.
"""

import os

import numpy as np

import concourse.bass as bass
import concourse.mybir as mybir
import concourse.tile as tile
from concourse import library_config

F32 = mybir.dt.float32
U16 = mybir.dt.uint16
I16 = mybir.dt.int16
_add_dep = bass._add_dep_helper

N, C, H, W = 2, 256, 128, 128
K = 5
S = 2
PAD = K // 2
SH, SW = H * S, W * S

N_CORES = 8
QH = H // 4          # 32 input rows per core
R_IN = QH + 2 * PAD  # 36 padded feature rows per core
N_I = QH             # 32 output row-pairs per core
YB = 8               # y rows per output DMA batch (4 i's)
RCH = 4              # feature rows per load chunk
NSL3 = 3 * K * 2 * 2  # uint16 scatter slots for the di 0..2 triple
NSL2 = 2 * K * 2 * 2  # uint16 scatter slots for the di 3..4 pair
NSL = NSL3 + NSL2
BTP = K * SW + 2      # bt yp-row length: K*SW fp32 payload + 2 fp32 pad


def _mi(x):
    return getattr(x, "ins", x)


def relocate_sync(pres, scats, posts):
    """Move the scatters' semaphore waits onto `pres` and updates onto
    `posts` (all chained in Pool-engine program order via nosync deps; Q7
    execution is strict FIFO per engine, so advancing waits and delaying
    updates across the group is sync-preserving).  Waits merge by max per
    semaphore, updates merge by sum."""
    def si_of(inst):
        si = inst.sync_info
        if si is None:
            return [], []
        return list(si.on_wait or []), list(si.on_update or [])

    wmax, uacc = {}, {}
    for s in scats:
        w, u = si_of(_mi(s))
        for x in w:
            assert x.sync_type == "semaphore" and x.wait_mode == "sem-ge-imm", x
            prev = wmax.get(x.id)
            if prev is None or x.wait_value > prev.wait_value:
                wmax[x.id] = x
        for x in u:
            assert x.sync_type == "semaphore" and x.update_mode in (
                "sem-inc", "sem-add-imm"), x
            prev = uacc.get(x.id)
            if prev is None:
                uacc[x.id] = mybir.SyncUpdate(
                    sync_type="semaphore", id=x.id, ant_name=x.ant_name,
                    update_mode="sem-add-imm", update_value=x.update_value)
            else:
                prev.update_value = prev.update_value + x.update_value
        _mi(s).sync_info = mybir.SyncInfo(on_wait=[], on_update=[])

    for carrier in pres:
        ci = _mi(carrier)
        cw, cu = si_of(ci)
        for w in cw:
            inc = wmax.pop(w.id, None)
            if inc is not None and inc.wait_value > w.wait_value:
                w.wait_value = inc.wait_value
        take = list(wmax.values())
        wmax.clear()
        ci.sync_info = mybir.SyncInfo(on_wait=cw + take, on_update=cu)
        break
    assert not wmax

    for carrier in posts:
        ci = _mi(carrier)
        cw, cu = si_of(ci)
        for u in cu:
            inc = uacc.pop(u.id, None)
            if inc is not None:
                u.update_value = u.update_value + inc.update_value
                u.update_mode = "sem-add-imm"
        take = list(uacc.values())
        uacc.clear()
        ci.sync_info = mybir.SyncInfo(on_wait=cw, on_update=cu + take)
        break
    assert not uacc


def split_sync(nc):
    """Enforce <=1 wait and <=1 update per instruction (this walrus build's
    events capacity), hoisting excess waits onto standalone same-engine
    sequencer NOPs placed immediately before (sync-equivalent).  Also hoists
    a wait that shares its semaphore with the instruction's own update."""
    for f in nc.m.functions:
        for b in f.blocks:
            lst = b.instructions
            i = 0
            while i < len(lst):
                inst = lst[i]
                si = getattr(inst, "sync_info", None)
                if si is None:
                    i += 1
                    continue
                w = list(si.on_wait or [])
                u = list(si.on_update or [])
                assert len(u) <= 1, (inst.name, u)
                uids = {x.id for x in u}
                conflict = any(x.id in uids for x in w) or (
                    w and any(x.update_mode == "sem-add-imm" for x in u))
                if len(w) <= 1 and not conflict:
                    i += 1
                    continue
                if (w and w[-1].id not in uids
                        and not any(x.update_mode == "sem-add-imm" for x in u)):
                    move, keep = w[:-1], w[-1:]
                else:
                    move, keep = w, []
                for wt in move:
                    nop = mybir.InstNoOp(
                        name=f"{inst.name}-ss{i}", text_hint="syncsplit")
                    nop.engine = inst.engine
                    nop.sync_info = mybir.SyncInfo(on_wait=[wt], on_update=[])
                    nc.register_instruction(nop, overwrite=True)
                    lst.insert(i, nop)
                    i += 1
                inst.sync_info = mybir.SyncInfo(on_wait=keep, on_update=u)
                i += 1


def host_gather(mask_shard: np.ndarray):
    """maskD[j', i, yp, di, dj, px] = mask[di*5+dj, 2i+yp, 2j'-2dj+4+px] (0 OOB)."""
    kk, ny, sw = mask_shard.shape
    ni = ny // 2
    m = mask_shard.reshape(K, K, ni, 2, sw)  # [di, dj, i, yp, x]
    d = np.zeros((128, ni, 2, K, K, 2), dtype=np.float32)
    for dj in range(K):
        for px in range(2):
            x = 2 * np.arange(128) - 2 * dj + 4 + px  # [128]
            valid = (x >= 0) & (x < sw)
            xc = np.clip(x, 0, sw - 1)
            sel = m[:, dj][:, :, :, xc]               # [di, i, yp, 128]
            sel = sel * valid[None, None, None, :]
            d[:, :, :, :, dj, px] = sel.transpose(3, 1, 2, 0)
    return np.ascontiguousarray(d)


def host_bidx():
    """Static scatter index tables (di 0..2 triple | di 3..4 pair) into a
    [K*SW fp32] dst row viewed as uint16."""
    def table(dis):
        idx = np.full((128, len(dis), K, 2, 2), -1, dtype=np.int16)
        for j in range(128):
            for dr, _ in enumerate(dis):
                for dj in range(K):
                    for px in range(2):
                        x = 2 * j - 2 * dj + 4 + px
                        if 0 <= x < SW:
                            idx[j, dr, dj, px, 0] = dr * 2 * SW + 2 * x
                            idx[j, dr, dj, px, 1] = dr * 2 * SW + 2 * x + 1
        return idx.reshape(128, -1)

    return np.ascontiguousarray(
        np.concatenate([table([0, 1, 2]), table([3, 4])], axis=1))


def build_program(n_i: int = N_I, r_in: int = R_IN, relocate: bool = True,
                  detect_races: bool = False):
    nc = bass.Bass(detect_race_conditions=detect_races)

    featt = nc.dram_tensor("featt", [128, r_in, C], F32, kind="ExternalInput")
    maskd = nc.dram_tensor(
        "maskd", [128, n_i, 2 * K * K * 2], F32, kind="ExternalInput"
    )
    bidx = nc.dram_tensor("bidx", [128, NSL], I16, kind="ExternalInput")
    out = nc.dram_tensor("out", [C, 2 * n_i, SW], F32, kind="ExternalOutput")

    assert r_in % RCH == 0
    groups = []

    with tile.TileContext(nc) as tc:
        with (
            tc.tile_pool(name="const", bufs=1) as constp,
            tc.tile_pool(name="ft", bufs=1) as ftp,
            tc.tile_pool(name="maskd", bufs=1) as mdp,
            tc.tile_pool(name="btile", bufs=4) as bp,
            tc.tile_pool(name="orow", bufs=3) as orowp,
            tc.tile_pool(name="mm", bufs=6, space="PSUM") as mmp,
        ):
            nc.gpsimd.load_library(library_config.local_scatter)
            bix = constp.tile([128, NSL], I16, tag="bix")
            nc.sync.dma_start(out=bix[:], in_=bidx[:])

            # maskD resident: [j', i, (yp, di, dj, px)] fp32
            md = mdp.tile([128, n_i, 2 * K * K * 2], F32)
            nc.sync.dma_start(out=md[:], in_=maskd[:])

            # ---- transposed feature rows ft[j', r, c] (host-pretransposed) ----
            ft = ftp.tile([128, r_in, C], F32)
            nc.sync.dma_start(out=ft[:], in_=featt[:])

            # ---- main loop over output row pairs ----
            IB = YB // 2
            for ib0 in range(0, n_i, IB):
                orow = orowp.tile([128, YB, 2, SW], F32, tag="orow")
                for ii in range(IB):
                    i = ib0 + ii
                    # B tiles for this row pair: [j', yp, (di, x) + pad]
                    bt = bp.tile([128, 2, BTP], F32, tag="bt")
                    pre = nc.gpsimd.memset(bt[:, 0, K * SW:BTP], 0.0)
                    if groups:
                        groups[-1][2] = pre  # pre also carries prev group's updates
                        _add_dep(_mi(pre), _mi(groups[-1][1][-1]), sync=False,
                                 reason="chain")
                    prev = pre
                    scats = []
                    for yp in range(2):
                        for lo, hi, i0, i1 in ((0, 3, 0, NSL3), (3, K, NSL3, NSL)):
                            sc = nc.gpsimd.local_scatter(
                                out_ap=bt[:, yp, lo * SW:hi * SW].bitcast(U16),
                                data_ap=md[
                                    :, i,
                                    (yp * K + lo) * K * 2:(yp * K + hi) * K * 2
                                ].bitcast(U16),
                                idxs_ap=bix[:, i0:i1],
                                channels=128,
                                num_elems=(hi - lo) * 2 * SW,
                                num_idxs=i1 - i0,
                            )
                            _add_dep(_mi(sc), _mi(prev), sync=False,
                                     reason="chain")
                            scats.append(sc)
                            prev = sc
                    groups.append([pre, scats, None])

                    for ch in range(2):
                        pm = mmp.tile([128, 2, SW], F32, tag="mm")
                        for di in range(K):
                            nc.tensor.matmul(
                                pm[:],
                                ft[:, i + di, ch * 128:(ch + 1) * 128],
                                bt[:, :, di * SW:(di + 1) * SW],
                                start=(di == 0),
                                stop=(di == K - 1),
                            )
                        if ch == 0:
                            nc.scalar.copy(
                                out=orow[:, 2 * ii:2 * ii + 2, ch, :],
                                in_=pm[:],
                            )
                        else:
                            nc.vector.tensor_copy(
                                orow[:, 2 * ii:2 * ii + 2, ch, :], pm[:],
                            )
                for ch in range(2):
                    dma_eng = nc.sync if ch == 0 else nc.scalar
                    dma_eng.dma_start(
                        out=out[ch * 128:(ch + 1) * 128,
                                2 * ib0:2 * ib0 + YB, :],
                        in_=orow[:, :, ch, :],
                    )
            term = nc.gpsimd.memset(bt[:, 1, K * SW:BTP], 0.0)
            _add_dep(_mi(term), _mi(groups[-1][1][-1]), sync=False,
                     reason="chain")
            groups[-1][2] = term

    if relocate:
        for pre, scats, post in groups:
            relocate_sync([pre], scats, [post])
        split_sync(nc)
    return nc


def finalize_for_hw(nc):
    assert mybir.codegen_inst_isa_subclasses(nc)
    return nc


_PROGRAM = None


def _get_program():
    global _PROGRAM
    if _PROGRAM is None:
        _PROGRAM = finalize_for_hw(build_program())
    return _PROGRAM


def kernel(features: np.ndarray, masks: np.ndarray) -> np.ndarray:
    from concourse.bass_utils import run_bass_kernel_spmd

    features = np.ascontiguousarray(features, dtype=np.float32)
    masks = np.ascontiguousarray(masks, dtype=np.float32)
    fpad = np.pad(features, ((0, 0), (0, 0), (PAD, PAD), (0, 0)))
    bix = host_bidx()

    in_maps = []
    for core in range(N_CORES):
        n, q = divmod(core, 4)
        ftt = fpad[n, :, QH * q:QH * q + R_IN, :].transpose(2, 1, 0)
        in_maps.append({
            "featt": np.ascontiguousarray(ftt),
            "maskd": host_gather(
                masks[n, :, 2 * N_I * q:2 * N_I * (q + 1), :]
            ).reshape(128, N_I, 2 * K * K * 2),
            "bidx": bix,
        })

    nc = _get_program()
    trace = os.environ.get("CARAFE_TRACE") == "1"
    res = run_bass_kernel_spmd(nc, in_maps, list(range(N_CORES)), trace=trace)
    kernel.last_results = res

    out = np.empty((N, C, SH, SW), dtype=np.float32)
    for core in range(N_CORES):
        n, q = divmod(core, 4)
        out[n, :, 2 * N_I * q:2 * N_I * (q + 1), :] = res.results[core]["out"]
    return out



# revision 3
# speedup vs baseline: 2.3204x; 2.3204x over previous
"""CARAFE D3: fp16 tensor-engine banded-matmul design.

out[c, y, x] = sum_di sum_dj fpad[c, y//2+di, x//2+dj] * m[di*5+dj, y, x]

For a fixed input row index i (covering output rows y=2i and 2i+1, which use
the same feature rows) and tap row di, the contribution over all (yp, x) is a
matmul contracting over the padded input column j' (128 lanes):

    out_i[c, (yp, x)] += sum_{j'} ftT[j', r=i+di, c] * B_di[j', (yp, x)]

where B_di[j', yp, x] = m[(di, dj), 2i+yp, x] at dj = j' - x//2 + 2 (banded,
5 diagonals per yp, zeros elsewhere).  The 5 di-taps accumulate in a full
512-wide PSUM bank.  All matmul operands are fp16 (1 cycle/row on PE vs 4
for fp32; PSUM accumulation stays fp32), and B tiles are built per (i, yp)
by GPSIMD local_scatter of fp16 mask values from a host-pregathered dense
tensor with a static index table (banded slot positions are y-independent).

The local_scatter extended-ISA instruction cannot carry semaphore ops through
this walrus build, so its sync is relocated onto adjacent Pool-engine memsets
(sound: Q7 execution is strict FIFO per engine), and a final pass splits any
instruction with more than one wait into standalone sequencer NOPs.
"""

import os

import numpy as np

import concourse.bass as bass
import concourse.mybir as mybir
import concourse.tile as tile
from concourse import library_config

F32 = mybir.dt.float32
F16 = mybir.dt.float16
I16 = mybir.dt.int16
_add_dep = bass._add_dep_helper

N, C, H, W = 2, 256, 128, 128
K = 5
S = 2
PAD = K // 2
SH, SW = H * S, W * S

N_CORES = 8
QH = H // 4          # 32 input rows per core
R_IN = QH + 2 * PAD  # 36 padded feature rows per core
N_I = QH             # 32 output row-pairs per core
YB = 8               # y rows per output DMA batch (4 i's)
NSL = K * K * 2      # 50 scatter slots (di, dj, px) per (j', i, yp)
BROW = 264           # bt row: 4 left pad + 256 + 4 right pad
FTCH = 12            # feature rows per load chunk


def _mi(x):
    return getattr(x, "ins", x)


def relocate_sync(pres, scats, posts):
    """Move the scatters' semaphore waits onto `pres` and updates onto
    `posts` (all chained in Pool-engine program order via nosync deps; Q7
    execution is strict FIFO per engine, so advancing waits and delaying
    updates across the group is sync-preserving).  Waits merge by max per
    semaphore, updates merge by sum."""
    def si_of(inst):
        si = inst.sync_info
        if si is None:
            return [], []
        return list(si.on_wait or []), list(si.on_update or [])

    wmax, uacc = {}, {}
    for s in scats:
        w, u = si_of(_mi(s))
        for x in w:
            assert x.sync_type == "semaphore" and x.wait_mode == "sem-ge-imm", x
            prev = wmax.get(x.id)
            if prev is None or x.wait_value > prev.wait_value:
                wmax[x.id] = x
        for x in u:
            assert x.sync_type == "semaphore" and x.update_mode in (
                "sem-inc", "sem-add-imm"), x
            prev = uacc.get(x.id)
            if prev is None:
                uacc[x.id] = mybir.SyncUpdate(
                    sync_type="semaphore", id=x.id, ant_name=x.ant_name,
                    update_mode="sem-add-imm", update_value=x.update_value)
            else:
                prev.update_value = prev.update_value + x.update_value
        _mi(s).sync_info = mybir.SyncInfo(on_wait=[], on_update=[])

    for carrier in pres:
        ci = _mi(carrier)
        cw, cu = si_of(ci)
        for w in cw:
            inc = wmax.pop(w.id, None)
            if inc is not None and inc.wait_value > w.wait_value:
                w.wait_value = inc.wait_value
        take = list(wmax.values())
        wmax.clear()
        ci.sync_info = mybir.SyncInfo(on_wait=cw + take, on_update=cu)
        break
    assert not wmax

    for carrier in posts:
        ci = _mi(carrier)
        cw, cu = si_of(ci)
        for u in cu:
            inc = uacc.pop(u.id, None)
            if inc is not None:
                u.update_value = u.update_value + inc.update_value
                u.update_mode = "sem-add-imm"
        take = list(uacc.values())
        uacc.clear()
        ci.sync_info = mybir.SyncInfo(on_wait=cw, on_update=cu + take)
        break
    assert not uacc


def split_sync(nc):
    """Enforce <=1 wait and <=1 update per instruction (this walrus build's
    events capacity), hoisting excess waits onto standalone same-engine
    sequencer NOPs placed immediately before (sync-equivalent).  Also hoists
    a wait that shares its semaphore with the instruction's own update."""
    for f in nc.m.functions:
        for b in f.blocks:
            lst = b.instructions
            i = 0
            while i < len(lst):
                inst = lst[i]
                si = getattr(inst, "sync_info", None)
                if si is None:
                    i += 1
                    continue
                w = list(si.on_wait or [])
                u = list(si.on_update or [])
                assert len(u) <= 1, (inst.name, u)
                uids = {x.id for x in u}
                conflict = any(x.id in uids for x in w) or (
                    w and any(x.update_mode == "sem-add-imm" for x in u))
                if len(w) <= 1 and not conflict:
                    i += 1
                    continue
                if (w and w[-1].id not in uids
                        and not any(x.update_mode == "sem-add-imm" for x in u)):
                    move, keep = w[:-1], w[-1:]
                else:
                    move, keep = w, []
                for wt in move:
                    nop = mybir.InstNoOp(
                        name=f"{inst.name}-ss{i}", text_hint="syncsplit")
                    nop.engine = inst.engine
                    nop.sync_info = mybir.SyncInfo(on_wait=[wt], on_update=[])
                    nc.register_instruction(nop, overwrite=True)
                    lst.insert(i, nop)
                    i += 1
                inst.sync_info = mybir.SyncInfo(on_wait=keep, on_update=u)
                i += 1


def host_maskq(mask_shard: np.ndarray):
    """maskq[j', i, yp, (di, dj, px)] = mask[di*5+dj, 2i+yp, 2j'-2dj+4+px]
    (0 where the x index is out of bounds), as fp16."""
    kk, ny, sw = mask_shard.shape
    ni = ny // 2
    m = mask_shard.reshape(K, K, ni, 2, sw)  # [di, dj, i, yp, x]
    d = np.zeros((128, ni, 2, K, K, 2), dtype=np.float16)
    for dj in range(K):
        for px in range(2):
            x = 2 * np.arange(128) - 2 * dj + 4 + px  # [128]
            valid = (x >= 0) & (x < sw)
            xc = np.clip(x, 0, sw - 1)
            sel = m[:, dj][:, :, :, xc]               # [di, i, yp, 128]
            sel = sel * valid[None, None, None, :]
            d[:, :, :, :, dj, px] = sel.transpose(3, 1, 2, 0).astype(np.float16)
    return np.ascontiguousarray(d.reshape(128, ni, 2, NSL))


def host_bidx():
    """Static scatter index table: slot (di, dj, px) of partition j' goes to
    bt position di*BROW + 2j' - 2dj + 8 + px (-1 where x OOB)."""
    idx = np.full((128, K, K, 2), -1, dtype=np.int16)
    for j in range(128):
        for di in range(K):
            for dj in range(K):
                for px in range(2):
                    x = 2 * j - 2 * dj + 4 + px
                    if 0 <= x < SW:
                        idx[j, di, dj, px] = di * BROW + x + 4
    return np.ascontiguousarray(idx.reshape(128, NSL))


def build_program(n_i: int = N_I, r_in: int = R_IN, relocate: bool = True,
                  detect_races: bool = False):
    nc = bass.Bass(detect_race_conditions=detect_races)

    featt = nc.dram_tensor("featt", [128, r_in, C], F16, kind="ExternalInput")
    maskq = nc.dram_tensor(
        "maskq", [128, n_i, 2, NSL], F16, kind="ExternalInput"
    )
    bidx = nc.dram_tensor("bidx", [128, NSL], I16, kind="ExternalInput")
    out = nc.dram_tensor("out", [C, 2 * n_i, SW], F32, kind="ExternalOutput")

    groups = []

    with tile.TileContext(nc) as tc:
        with (
            tc.tile_pool(name="const", bufs=1) as constp,
            tc.tile_pool(name="ft", bufs=1) as ftp,
            tc.tile_pool(name="maskq", bufs=1) as mdp,
            tc.tile_pool(name="btile", bufs=4) as bp,
            tc.tile_pool(name="orow", bufs=3) as orowp,
            tc.tile_pool(name="mm", bufs=6, space="PSUM") as mmp,
        ):
            nc.gpsimd.load_library(library_config.local_scatter)
            bix = constp.tile([128, NSL], I16, tag="bix")
            nc.sync.dma_start(out=bix[:], in_=bidx[:])

            # mask values resident: [j', i, yp, (di, dj, px)] fp16
            md = mdp.tile([128, n_i, 2, NSL], F16)
            nc.sync.dma_start(out=md[:], in_=maskq[:])

            # transposed feature rows ft[j', r, c] (host-pretransposed), fp16
            ft = ftp.tile([128, r_in, C], F16)
            for r0 in range(0, r_in, FTCH):
                r1 = min(r0 + FTCH, r_in)
                nc.sync.dma_start(
                    out=ft[:, r0:r1, :], in_=featt[:, r0:r1, :]
                )

            # ---- main loop over output row pairs ----
            IB = YB // 2
            for ib0 in range(0, n_i, IB):
                orow = orowp.tile([128, YB, 2, SW], F32, tag="orow")
                for ii in range(IB):
                    i = ib0 + ii
                    # B tile for this row pair: [j', yp, (di, q) + pad]
                    bt = bp.tile([128, 2, K * BROW + 2], F16, tag="bt")
                    pre = nc.gpsimd.memset(bt[:, 0, K * BROW:], 0.0)
                    if groups:
                        groups[-1][2] = pre  # pre also carries prev updates
                        _add_dep(_mi(pre), _mi(groups[-1][1][-1]), sync=False,
                                 reason="chain")
                    prev = pre
                    scats = []
                    for yp in range(2):
                        sc = nc.gpsimd.local_scatter(
                            out_ap=bt[:, yp, 0:K * BROW],
                            data_ap=md[:, i, yp, :],
                            idxs_ap=bix[:],
                            channels=128,
                            num_elems=K * BROW,
                            num_idxs=NSL,
                        )
                        _add_dep(_mi(sc), _mi(prev), sync=False,
                                 reason="chain")
                        scats.append(sc)
                        prev = sc
                    groups.append([pre, scats, None])

                    for ch in range(2):
                        pm = mmp.tile([128, 2, SW], F32, tag="mm")
                        for di in range(K):
                            nc.tensor.matmul(
                                pm[:],
                                ft[:, i + di, ch * 128:(ch + 1) * 128],
                                bt[:, :, di * BROW + 4:di * BROW + 4 + SW],
                                start=(di == 0),
                                stop=(di == K - 1),
                            )
                        if ch == 0:
                            nc.scalar.copy(
                                out=orow[:, 2 * ii:2 * ii + 2, ch, :],
                                in_=pm[:],
                            )
                        else:
                            nc.vector.tensor_copy(
                                orow[:, 2 * ii:2 * ii + 2, ch, :], pm[:],
                            )
                for ch in range(2):
                    dma_eng = nc.sync if ch == 0 else nc.scalar
                    dma_eng.dma_start(
                        out=out[ch * 128:(ch + 1) * 128,
                                2 * ib0:2 * ib0 + YB, :],
                        in_=orow[:, :, ch, :],
                    )
            term = nc.gpsimd.memset(bt[:, 1, K * BROW:], 0.0)
            _add_dep(_mi(term), _mi(groups[-1][1][-1]), sync=False,
                     reason="chain")
            groups[-1][2] = term

    if relocate:
        for pre, scats, post in groups:
            relocate_sync([pre], scats, [post])
        split_sync(nc)
    return nc


def finalize_for_hw(nc):
    assert mybir.codegen_inst_isa_subclasses(nc)
    return nc


_PROGRAM = None


def _get_program():
    global _PROGRAM
    if _PROGRAM is None:
        _PROGRAM = finalize_for_hw(build_program())
    return _PROGRAM


def kernel(features: np.ndarray, masks: np.ndarray) -> np.ndarray:
    from concourse.bass_utils import run_bass_kernel_spmd

    features = np.ascontiguousarray(features, dtype=np.float32)
    masks = np.ascontiguousarray(masks, dtype=np.float32)
    fpad = np.pad(features, ((0, 0), (0, 0), (PAD, PAD), (0, 0)))
    bix = host_bidx()

    in_maps = []
    for core in range(N_CORES):
        n, q = divmod(core, 4)
        ftt = fpad[n, :, QH * q:QH * q + R_IN, :].transpose(2, 1, 0)
        in_maps.append({
            "featt": np.ascontiguousarray(ftt.astype(np.float16)),
            "maskq": host_maskq(
                masks[n, :, 2 * N_I * q:2 * N_I * (q + 1), :]
            ),
            "bidx": bix,
        })

    nc = _get_program()
    trace = os.environ.get("CARAFE_TRACE") == "1"
    res = run_bass_kernel_spmd(nc, in_maps, list(range(N_CORES)), trace=trace)
    kernel.last_results = res

    out = np.empty((N, C, SH, SW), dtype=np.float32)
    for core in range(N_CORES):
        n, q = divmod(core, 4)
        out[n, :, 2 * N_I * q:2 * N_I * (q + 1), :] = res.results[core]["out"]
    return out


# revision 6
# speedup vs baseline: 3.2737x; 1.4108x over previous
"""CARAFE D4: fp16 banded-matmul with split B-tile construction.

out[c, y, x] = sum_di sum_dj fpad[c, y//2+di, x//2+dj] * m[di*5+dj, y, x]

For a fixed input row index i (covering output rows y=2i and 2i+1, which use
the same feature rows) and tap row di, the contribution over all (yp, x) is a
matmul contracting over the padded input column j' (128 lanes):

    out_i[c, (yp, x)] += sum_{j'} ftT[j', r=i+di, c] * B_di[j', (yp, x)]

where B_di[j', yp, x] = m[(di, dj), 2i+yp, x] at dj = j' - x//2 + 2 (banded,
5 diagonals per yp, zeros elsewhere).  The 5 di-taps accumulate in a full
512-wide PSUM bank.  All matmul operands are fp16 (1 cycle/row on PE vs 4
for fp32; PSUM accumulation stays fp32).

B tiles are built two ways, balancing GPSIMD against DMA bandwidth:
 - the yp=0 slabs and the yp=1/di=4 slab by GPSIMD local_scatter of fp16
   mask values from a host-pregathered tensor with a static index table
   (the banded slot positions are y-independent), and
 - the yp=1 di=0..3 slabs by a plain dense DMA of host-prebanded rows
   (zeros included) straight into the B tile.

The local_scatter extended-ISA instruction cannot carry semaphore ops through
this walrus build, so its sync is relocated onto adjacent Pool-engine memsets
(sound: Q7 execution is strict FIFO per engine), and a final pass splits any
instruction with more than one wait into standalone sequencer NOPs.
"""

import os

import numpy as np

import concourse.bass as bass
import concourse.mybir as mybir
import concourse.tile as tile
from concourse import library_config

F32 = mybir.dt.float32
F16 = mybir.dt.float16
I16 = mybir.dt.int16
_add_dep = bass._add_dep_helper

N, C, H, W = 2, 256, 128, 128
K = 5
S = 2
PAD = K // 2
SH, SW = H * S, W * S

N_CORES = 8
QH = H // 4          # 32 input rows per core
R_IN = QH + 2 * PAD  # 36 padded feature rows per core
N_I = QH             # 32 output row-pairs per core
YB = 8               # y rows per output DMA batch (4 i's)
NSL0 = K * K * 2     # 50 scatter slots (di, dj, px) for the yp=0 slabs
NSL1 = K * 2         # 10 scatter slots (dj, px) for the yp=1 di=4 slab
NSL = NSL0 + NSL1
KDMA = 4             # yp=1 slabs 0..KDMA-1 arrive by dense prebanded DMA
FTCH = 12            # feature rows per load chunk


def _mi(x):
    return getattr(x, "ins", x)


def relocate_sync(pres, scats, posts):
    """Move the scatters' semaphore waits onto `pres` and updates onto
    `posts` (all chained in Pool-engine program order via nosync deps; Q7
    execution is strict FIFO per engine, so advancing waits and delaying
    updates across the group is sync-preserving).  Waits merge by max per
    semaphore, updates merge by sum."""
    def si_of(inst):
        si = inst.sync_info
        if si is None:
            return [], []
        return list(si.on_wait or []), list(si.on_update or [])

    wmax, uacc = {}, {}
    for s in scats:
        w, u = si_of(_mi(s))
        for x in w:
            assert x.sync_type == "semaphore" and x.wait_mode == "sem-ge-imm", x
            prev = wmax.get(x.id)
            if prev is None or x.wait_value > prev.wait_value:
                wmax[x.id] = x
        for x in u:
            assert x.sync_type == "semaphore" and x.update_mode in (
                "sem-inc", "sem-add-imm"), x
            prev = uacc.get(x.id)
            if prev is None:
                uacc[x.id] = mybir.SyncUpdate(
                    sync_type="semaphore", id=x.id, ant_name=x.ant_name,
                    update_mode="sem-add-imm", update_value=x.update_value)
            else:
                prev.update_value = prev.update_value + x.update_value
        _mi(s).sync_info = mybir.SyncInfo(on_wait=[], on_update=[])

    for carrier in pres:
        ci = _mi(carrier)
        cw, cu = si_of(ci)
        for w in cw:
            inc = wmax.pop(w.id, None)
            if inc is not None and inc.wait_value > w.wait_value:
                w.wait_value = inc.wait_value
        take = list(wmax.values())
        wmax.clear()
        ci.sync_info = mybir.SyncInfo(on_wait=cw + take, on_update=cu)
        break
    assert not wmax

    for carrier in posts:
        ci = _mi(carrier)
        cw, cu = si_of(ci)
        for u in cu:
            inc = uacc.pop(u.id, None)
            if inc is not None:
                u.update_value = u.update_value + inc.update_value
                u.update_mode = "sem-add-imm"
        take = list(uacc.values())
        uacc.clear()
        ci.sync_info = mybir.SyncInfo(on_wait=cw, on_update=cu + take)
        break
    assert not uacc


def split_sync(nc):
    """Enforce <=1 wait and <=1 update per instruction (this walrus build's
    events capacity), hoisting excess waits onto standalone same-engine
    sequencer NOPs placed immediately before (sync-equivalent).  Also hoists
    a wait that shares its semaphore with the instruction's own update."""
    for f in nc.m.functions:
        for b in f.blocks:
            lst = b.instructions
            i = 0
            while i < len(lst):
                inst = lst[i]
                si = getattr(inst, "sync_info", None)
                if si is None:
                    i += 1
                    continue
                w = list(si.on_wait or [])
                u = list(si.on_update or [])
                assert len(u) <= 1, (inst.name, u)
                uids = {x.id for x in u}
                conflict = any(x.id in uids for x in w) or (
                    w and any(x.update_mode == "sem-add-imm" for x in u))
                if len(w) <= 1 and not conflict:
                    i += 1
                    continue
                if (w and w[-1].id not in uids
                        and not any(x.update_mode == "sem-add-imm" for x in u)):
                    move, keep = w[:-1], w[-1:]
                else:
                    move, keep = w, []
                for wt in move:
                    nop = mybir.InstNoOp(
                        name=f"{inst.name}-ss{i}", text_hint="syncsplit")
                    nop.engine = inst.engine
                    nop.sync_info = mybir.SyncInfo(on_wait=[wt], on_update=[])
                    nc.register_instruction(nop, overwrite=True)
                    lst.insert(i, nop)
                    i += 1
                inst.sync_info = mybir.SyncInfo(on_wait=keep, on_update=u)
                i += 1


def _gather_slots(m, yp, dis):
    """slots[j', i, (di in dis, dj, px)] = m[di*5+dj, 2i+yp, 2j'-2dj+4+px]
    (0 where x is out of bounds), fp16.  m: [K*K, 2*ni, SW] fp32."""
    kk, ny, sw = m.shape
    ni = ny // 2
    mr = m.reshape(K, K, ni, 2, sw)  # [di, dj, i, yp, x]
    d = np.zeros((128, ni, len(dis), K, 2), dtype=np.float16)
    for dj in range(K):
        for px in range(2):
            x = 2 * np.arange(128) - 2 * dj + 4 + px  # [128]
            valid = (x >= 0) & (x < sw)
            xc = np.clip(x, 0, sw - 1)
            sel = mr[dis, dj][:, :, yp][:, :, xc]     # [dis, i, 128]
            sel = sel * valid[None, None, :]
            d[:, :, :, dj, px] = sel.transpose(2, 1, 0).astype(np.float16)
    return d.reshape(128, ni, len(dis) * K * 2)


def host_maskq(mask_shard: np.ndarray):
    """Concatenated scatter payloads: yp=0 all di (50) | yp=1 di=4 (10)."""
    a = _gather_slots(mask_shard, 0, list(range(K)))
    b = _gather_slots(mask_shard, 1, [K - 1])
    return np.ascontiguousarray(np.concatenate([a, b], axis=2))


def host_prebanded(mask_shard: np.ndarray):
    """Dense banded yp=1 slabs for di=0..KDMA-1:
    pb[j', i, di, x] = m[di*5+dj, 2i+1, x] at x = 2j'-2dj+4+px, else 0."""
    kk, ny, sw = mask_shard.shape
    ni = ny // 2
    mr = mask_shard.reshape(K, K, ni, 2, sw)
    pb = np.zeros((128, ni, KDMA, SW), dtype=np.float16)
    j = np.arange(128)
    for dj in range(K):
        for px in range(2):
            x = 2 * j - 2 * dj + 4 + px
            valid = (x >= 0) & (x < sw)
            jv = j[valid]
            pb[jv, :, :, x[valid]] = (
                mr[:KDMA, dj, :, 1][:, :, x[valid]]
                .transpose(2, 1, 0).astype(np.float16)
            )
    return np.ascontiguousarray(pb)


def host_bidx():
    """Static scatter index tables.  First 50 slots: yp=0, position
    di*SW + 2j' - 2dj + 4 + px.  Last 10: yp=1 di=4, position (K-1)*SW + x.
    -1 where x OOB."""
    idx = np.full((128, NSL), -1, dtype=np.int16)
    for j in range(128):
        s = 0
        for di in range(K):
            for dj in range(K):
                for px in range(2):
                    x = 2 * j - 2 * dj + 4 + px
                    if 0 <= x < SW:
                        idx[j, s] = di * SW + x
                    s += 1
        for dj in range(K):
            for px in range(2):
                x = 2 * j - 2 * dj + 4 + px
                if 0 <= x < SW:
                    idx[j, s] = x  # relative to the di=K-1 slab out_ap
                s += 1
    return np.ascontiguousarray(idx)


def build_program(n_i: int = N_I, r_in: int = R_IN, relocate: bool = True,
                  detect_races: bool = False):
    nc = bass.Bass(detect_race_conditions=detect_races)

    featt = nc.dram_tensor("featt", [128, r_in, C], F16, kind="ExternalInput")
    maskq = nc.dram_tensor(
        "maskq", [128, n_i, NSL], F16, kind="ExternalInput"
    )
    preb = nc.dram_tensor(
        "preb", [128, n_i, KDMA, SW], F16, kind="ExternalInput"
    )
    bidx = nc.dram_tensor("bidx", [128, NSL], I16, kind="ExternalInput")
    out = nc.dram_tensor("out", [C, 2 * n_i, SW], F32, kind="ExternalOutput")

    groups = []
    BROW = K * SW  # 1280 elements per yp

    with tile.TileContext(nc) as tc:
        with (
            tc.tile_pool(name="const", bufs=1) as constp,
            tc.tile_pool(name="ft", bufs=1) as ftp,
            tc.tile_pool(name="maskq", bufs=1) as mdp,
            tc.tile_pool(name="btile", bufs=4) as bp,
            tc.tile_pool(name="orow", bufs=3) as orowp,
            tc.tile_pool(name="mm", bufs=6, space="PSUM") as mmp,
        ):
            nc.gpsimd.load_library(library_config.local_scatter)
            bix = constp.tile([128, NSL], I16, tag="bix")
            nc.sync.dma_start(out=bix[:], in_=bidx[:])

            # scatter payloads resident: [j', i, slots] fp16
            md = mdp.tile([128, n_i, NSL], F16)
            nc.sync.dma_start(out=md[:], in_=maskq[:])

            # transposed feature rows ft[j', r, c] (host-pretransposed), fp16
            ft = ftp.tile([128, r_in, C], F16)
            for r0 in range(0, r_in, FTCH):
                r1 = min(r0 + FTCH, r_in)
                nc.sync.dma_start(
                    out=ft[:, r0:r1, :], in_=featt[:, r0:r1, :]
                )

            # ---- main loop over output row pairs ----
            IB = YB // 2
            for ib0 in range(0, n_i, IB):
                orow = orowp.tile([128, YB, 2, SW], F32, tag="orow")
                for ii in range(IB):
                    i = ib0 + ii
                    # B tile: [j', yp, (di, x) + 2 pad]
                    bt = bp.tile([128, 2, BROW + 2], F16, tag="bt")
                    # yp=1 di<KDMA slabs: dense prebanded DMA
                    dma_eng = nc.sync if i % 2 == 0 else nc.scalar
                    dma_eng.dma_start(
                        out=bt[:, 1, 0:KDMA * SW], in_=preb[:, i]
                    )
                    pre = nc.gpsimd.memset(bt[:, 0, BROW:], 0.0)
                    if groups:
                        groups[-1][2] = pre  # pre also carries prev updates
                        _add_dep(_mi(pre), _mi(groups[-1][1][-1]), sync=False,
                                 reason="chain")
                    prev = pre
                    scats = []
                    for out_ap, s0, s1 in (
                        (bt[:, 0, 0:BROW], 0, NSL0),
                        (bt[:, 1, KDMA * SW:BROW], NSL0, NSL),
                    ):
                        sc = nc.gpsimd.local_scatter(
                            out_ap=out_ap,
                            data_ap=md[:, i, s0:s1],
                            idxs_ap=bix[:, s0:s1],
                            channels=128,
                            num_elems=(BROW, (K - KDMA) * SW)[s0 > 0],
                            num_idxs=s1 - s0,
                        )
                        _add_dep(_mi(sc), _mi(prev), sync=False,
                                 reason="chain")
                        scats.append(sc)
                        prev = sc
                    groups.append([pre, scats, None])

                    for ch in range(2):
                        pm = mmp.tile([128, 2, SW], F32, tag="mm")
                        for di in range(K):
                            nc.tensor.matmul(
                                pm[:],
                                ft[:, i + di, ch * 128:(ch + 1) * 128],
                                bt[:, :, di * SW:(di + 1) * SW],
                                start=(di == 0),
                                stop=(di == K - 1),
                            )
                        if ch == 0:
                            nc.scalar.copy(
                                out=orow[:, 2 * ii:2 * ii + 2, ch, :],
                                in_=pm[:],
                            )
                        else:
                            nc.vector.tensor_copy(
                                orow[:, 2 * ii:2 * ii + 2, ch, :], pm[:],
                            )
                for ch in range(2):
                    dma_eng = nc.sync if ch == 0 else nc.scalar
                    dma_eng.dma_start(
                        out=out[ch * 128:(ch + 1) * 128,
                                2 * ib0:2 * ib0 + YB, :],
                        in_=orow[:, :, ch, :],
                    )
            term = nc.gpsimd.memset(bt[:, 1, BROW:], 0.0)
            _add_dep(_mi(term), _mi(groups[-1][1][-1]), sync=False,
                     reason="chain")
            groups[-1][2] = term

    if relocate:
        for pre, scats, post in groups:
            relocate_sync([pre], scats, [post])
        split_sync(nc)
    return nc


def finalize_for_hw(nc):
    assert mybir.codegen_inst_isa_subclasses(nc)
    return nc


_PROGRAM = None


def _get_program():
    global _PROGRAM
    if _PROGRAM is None:
        _PROGRAM = finalize_for_hw(build_program())
    return _PROGRAM


def kernel(features: np.ndarray, masks: np.ndarray) -> np.ndarray:
    from concourse.bass_utils import run_bass_kernel_spmd

    features = np.ascontiguousarray(features, dtype=np.float32)
    masks = np.ascontiguousarray(masks, dtype=np.float32)
    fpad = np.pad(features, ((0, 0), (0, 0), (PAD, PAD), (0, 0)))
    bix = host_bidx()

    in_maps = []
    for core in range(N_CORES):
        n, q = divmod(core, 4)
        ftt = fpad[n, :, QH * q:QH * q + R_IN, :].transpose(2, 1, 0)
        mshard = masks[n, :, 2 * N_I * q:2 * N_I * (q + 1), :]
        in_maps.append({
            "featt": np.ascontiguousarray(ftt.astype(np.float16)),
            "maskq": host_maskq(mshard),
            "preb": host_prebanded(mshard),
            "bidx": bix,
        })

    nc = _get_program()
    trace = os.environ.get("CARAFE_TRACE") == "1"
    res = run_bass_kernel_spmd(nc, in_maps, list(range(N_CORES)), trace=trace)
    kernel.last_results = res

    out = np.empty((N, C, SH, SW), dtype=np.float32)
    for core in range(N_CORES):
        n, q = divmod(core, 4)
        out[n, :, 2 * N_I * q:2 * N_I * (q + 1), :] = res.results[core]["out"]
    return out


# revision 19
# speedup vs baseline: 4.0990x; 1.2521x over previous
"""CARAFE D4: fp16 banded-matmul with split B-tile construction.

out[c, y, x] = sum_di sum_dj fpad[c, y//2+di, x//2+dj] * m[di*5+dj, y, x]

For a fixed input row index i (covering output rows y=2i and 2i+1, which use
the same feature rows) and tap row di, the contribution over all (yp, x) is a
matmul contracting over the padded input column j' (128 lanes):

    out_i[c, (yp, x)] += sum_{j'} ftT[j', r=i+di, c] * B_di[j', (yp, x)]

where B_di[j', yp, x] = m[(di, dj), 2i+yp, x] at dj = j' - x//2 + 2 (banded,
5 diagonals per yp, zeros elsewhere).  The 5 di-taps accumulate in a full
512-wide PSUM bank.  All matmul operands are fp16 (1 cycle/row on PE vs 4
for fp32; PSUM accumulation stays fp32).

B tiles are built two ways, balancing GPSIMD against DMA bandwidth:
 - the yp=0 slabs and the yp=1/di=4 slab by GPSIMD local_scatter of fp16
   mask values from a host-pregathered tensor with a static index table
   (the banded slot positions are y-independent), and
 - the yp=1 di=0..3 slabs by a plain dense DMA of host-prebanded rows
   (zeros included) straight into the B tile.

The local_scatter extended-ISA instruction cannot carry semaphore ops through
this walrus build, so its sync is relocated onto adjacent Pool-engine memsets
(sound: Q7 execution is strict FIFO per engine), and a final pass splits any
instruction with more than one wait into standalone sequencer NOPs.
"""

import os

import numpy as np

import concourse.bass as bass
import concourse.mybir as mybir
import concourse.tile as tile
from concourse import library_config

F32 = mybir.dt.float32
F16 = mybir.dt.float16
I16 = mybir.dt.int16
_add_dep = bass._add_dep_helper

N, C, H, W = 2, 256, 128, 128
K = 5
S = 2
PAD = K // 2
SH, SW = H * S, W * S

N_CORES = 8
QH = H // 4          # 32 input rows per core
R_IN = QH + 2 * PAD  # 36 padded feature rows per core
N_I = QH             # 32 output row-pairs per core
YB = 4               # y rows per output DMA batch (2 i's)
NSL = K * K * 2      # 50 scatter slots (di, dj, px) for the yp=0 slabs
KDMA = K             # all yp=1 slabs arrive by dense prebanded DMA
FTCH = 12            # feature rows per load chunk


def _mi(x):
    return getattr(x, "ins", x)


def relocate_sync(pres, scats, posts):
    """Move the scatters' semaphore waits onto `pres` and updates onto
    `posts` (all chained in Pool-engine program order via nosync deps; Q7
    execution is strict FIFO per engine, so advancing waits and delaying
    updates across the group is sync-preserving).  Waits merge by max per
    semaphore, updates merge by sum."""
    def si_of(inst):
        si = inst.sync_info
        if si is None:
            return [], []
        return list(si.on_wait or []), list(si.on_update or [])

    wmax, uacc = {}, {}
    for s in scats:
        w, u = si_of(_mi(s))
        for x in w:
            assert x.sync_type == "semaphore" and x.wait_mode == "sem-ge-imm", x
            prev = wmax.get(x.id)
            if prev is None or x.wait_value > prev.wait_value:
                wmax[x.id] = x
        for x in u:
            assert x.sync_type == "semaphore" and x.update_mode in (
                "sem-inc", "sem-add-imm"), x
            prev = uacc.get(x.id)
            if prev is None:
                uacc[x.id] = mybir.SyncUpdate(
                    sync_type="semaphore", id=x.id, ant_name=x.ant_name,
                    update_mode="sem-add-imm", update_value=x.update_value)
            else:
                prev.update_value = prev.update_value + x.update_value
        _mi(s).sync_info = mybir.SyncInfo(on_wait=[], on_update=[])

    for carrier in pres:
        ci = _mi(carrier)
        cw, cu = si_of(ci)
        for w in cw:
            inc = wmax.pop(w.id, None)
            if inc is not None and inc.wait_value > w.wait_value:
                w.wait_value = inc.wait_value
        take = list(wmax.values())
        wmax.clear()
        ci.sync_info = mybir.SyncInfo(on_wait=cw + take, on_update=cu)
        break
    assert not wmax

    for carrier in posts:
        ci = _mi(carrier)
        cw, cu = si_of(ci)
        for u in cu:
            inc = uacc.pop(u.id, None)
            if inc is not None:
                u.update_value = u.update_value + inc.update_value
                u.update_mode = "sem-add-imm"
        take = list(uacc.values())
        uacc.clear()
        ci.sync_info = mybir.SyncInfo(on_wait=cw, on_update=cu + take)
        break
    assert not uacc


def split_sync(nc):
    """Enforce <=1 wait and <=1 update per instruction (this walrus build's
    events capacity), hoisting excess waits onto standalone same-engine
    sequencer NOPs placed immediately before (sync-equivalent).  Also hoists
    a wait that shares its semaphore with the instruction's own update."""
    for f in nc.m.functions:
        for b in f.blocks:
            lst = b.instructions
            i = 0
            while i < len(lst):
                inst = lst[i]
                si = getattr(inst, "sync_info", None)
                if si is None:
                    i += 1
                    continue
                w = list(si.on_wait or [])
                u = list(si.on_update or [])
                assert len(u) <= 1, (inst.name, u)
                uids = {x.id for x in u}
                conflict = any(x.id in uids for x in w) or (
                    w and any(x.update_mode == "sem-add-imm" for x in u))
                if len(w) <= 1 and not conflict:
                    i += 1
                    continue
                if (w and w[-1].id not in uids
                        and not any(x.update_mode == "sem-add-imm" for x in u)):
                    move, keep = w[:-1], w[-1:]
                else:
                    move, keep = w, []
                for wt in move:
                    nop = mybir.InstNoOp(
                        name=f"{inst.name}-ss{i}", text_hint="syncsplit")
                    nop.engine = inst.engine
                    nop.sync_info = mybir.SyncInfo(on_wait=[wt], on_update=[])
                    nc.register_instruction(nop, overwrite=True)
                    lst.insert(i, nop)
                    i += 1
                inst.sync_info = mybir.SyncInfo(on_wait=keep, on_update=u)
                i += 1


def _gather_slots(m, yp, dis):
    """slots[j', i, (di in dis, dj, px)] = m[di*5+dj, 2i+yp, 2j'-2dj+4+px]
    (0 where x is out of bounds), fp16.  m: [K*K, 2*ni, SW] fp32."""
    kk, ny, sw = m.shape
    ni = ny // 2
    mr = m.reshape(K, K, ni, 2, sw)  # [di, dj, i, yp, x]
    d = np.zeros((128, ni, len(dis), K, 2), dtype=np.float16)
    for dj in range(K):
        for px in range(2):
            x = 2 * np.arange(128) - 2 * dj + 4 + px  # [128]
            valid = (x >= 0) & (x < sw)
            xc = np.clip(x, 0, sw - 1)
            sel = mr[dis, dj][:, :, yp][:, :, xc]     # [dis, i, 128]
            sel = sel * valid[None, None, :]
            d[:, :, :, dj, px] = sel.transpose(2, 1, 0).astype(np.float16)
    return d.reshape(128, ni, len(dis) * K * 2)


def host_maskq(mask_shard: np.ndarray):
    """Scatter payloads for the yp=0 slabs (50 slots)."""
    return np.ascontiguousarray(_gather_slots(mask_shard, 0, list(range(K))))


def host_banded(mask_shard: np.ndarray, yp: int, i0: int, i1: int):
    """Dense banded slabs pb[j', i-i0, di, x] = m[di*5+dj, 2i+yp, x] at
    x = 2j'-2dj+4+px, else 0 (i in [i0, i1))."""
    kk, ny, sw = mask_shard.shape
    ni = ny // 2
    mr = mask_shard.reshape(K, K, ni, 2, sw)
    pb = np.zeros((128, i1 - i0, K, SW), dtype=np.float16)
    j = np.arange(128)
    for dj in range(K):
        for px in range(2):
            x = 2 * j - 2 * dj + 4 + px
            valid = (x >= 0) & (x < sw)
            jv = j[valid]
            pb[jv, :, :, x[valid]] = (
                mr[:, dj, i0:i1, yp][:, :, x[valid]]
                .transpose(2, 1, 0).astype(np.float16)
            )
    return pb


NPF = 2  # first row-pairs shipped fully prebanded (both yp, no scatter)


def host_prebanded(mask_shard: np.ndarray):
    return np.ascontiguousarray(host_banded(mask_shard, 1, 0, N_I))


def host_prebf(mask_shard: np.ndarray):
    a = host_banded(mask_shard, 0, 0, NPF)
    b = host_banded(mask_shard, 1, 0, NPF)
    return np.ascontiguousarray(np.stack([a, b], axis=2))  # [128, NPF, 2, K, SW]


def host_bidx():
    """Static scatter index table: slot (di, dj, px) of partition j' goes
    to position di*SW + 2j' - 2dj + 4 + px (-1 where x OOB)."""
    idx = np.full((128, NSL), -1, dtype=np.int16)
    for j in range(128):
        s = 0
        for di in range(K):
            for dj in range(K):
                for px in range(2):
                    x = 2 * j - 2 * dj + 4 + px
                    if 0 <= x < SW:
                        idx[j, s] = di * SW + x
                    s += 1
    return np.ascontiguousarray(idx)


def build_program(n_i: int = N_I, r_in: int = R_IN, relocate: bool = True,
                  detect_races: bool = False, yb: int = YB, lookahead: int = 6,
                  bt_bufs: int = 6, orow_bufs: int = 3, mm_bufs: int = 6):
    nc = bass.Bass(detect_race_conditions=detect_races)

    featt = nc.dram_tensor("featt", [128, r_in, C], F16, kind="ExternalInput")
    maskq = nc.dram_tensor(
        "maskq", [128, n_i, NSL], F16, kind="ExternalInput"
    )
    preb = nc.dram_tensor(
        "preb", [128, n_i, KDMA, SW], F16, kind="ExternalInput"
    )
    bidx = nc.dram_tensor("bidx", [128, NSL], I16, kind="ExternalInput")
    out = nc.dram_tensor("out", [C, 2 * n_i, SW], F16, kind="ExternalOutput")

    groups = []
    BROW = K * SW  # 1280 elements per yp

    with tile.TileContext(nc) as tc:
        with (
            tc.tile_pool(name="const", bufs=1) as constp,
            tc.tile_pool(name="ft", bufs=1) as ftp,
            tc.tile_pool(name="maskq", bufs=1) as mdp,
            tc.tile_pool(name="btile", bufs=bt_bufs) as bp,
            tc.tile_pool(name="orow", bufs=orow_bufs) as orowp,
            tc.tile_pool(name="mm", bufs=mm_bufs, space="PSUM") as mmp,
        ):
            nc.gpsimd.load_library(library_config.local_scatter)
            bix = constp.tile([128, NSL], I16, tag="bix")
            nc.scalar.dma_start(out=bix[:], in_=bidx[:])

            # B tiles are allocated LOOKAHEAD iterations early so their
            # yp=1 prebanded DMA can be issued ahead of the compute wave.
            LOOKAHEAD = lookahead
            bts = {}

            def issue_preb(i, eng=None):
                bt = bp.tile([128, 2, K * SW + 2], F16, tag="bt")
                if eng is None:
                    eng = nc.scalar if i % 2 == 0 else nc.sync
                eng.dma_start(out=bt[:, 1, 0:KDMA * SW], in_=preb[:, i])
                bts[i] = bt

            # scatter payloads resident: [j', i, slots] fp16, loaded in
            # chunks so the first scatters can start early
            md = mdp.tile([128, n_i, NSL], F16)
            ft = ftp.tile([128, r_in, C], F16)
            nc.sync.dma_start(out=md[:, 0:4], in_=maskq[:, 0:4])
            nc.sync.dma_start(out=ft[:, 0:6, :], in_=featt[:, 0:6, :])
            issue_preb(0)
            issue_preb(1)
            nc.sync.dma_start(out=md[:, 4:8], in_=maskq[:, 4:8])
            issue_preb(2)
            nc.sync.dma_start(out=ft[:, 6:12, :], in_=featt[:, 6:12, :])
            issue_preb(3)
            nc.sync.dma_start(out=md[:, 8:12], in_=maskq[:, 8:12])
            for i in range(4, LOOKAHEAD):
                issue_preb(i)
            nc.sync.dma_start(out=md[:, 12:], in_=maskq[:, 12:])
            for r0 in range(12, r_in, FTCH):
                r1 = min(r0 + FTCH, r_in)
                nc.sync.dma_start(
                    out=ft[:, r0:r1, :], in_=featt[:, r0:r1, :]
                )

            # ---- main loop over output row pairs ----
            IB = yb // 2
            for ib0 in range(0, n_i, IB):
                orow = orowp.tile([128, yb, 2, SW], F16, tag="orow")
                for ii in range(IB):
                    i = ib0 + ii
                    bt = bts.pop(i)
                    if i + LOOKAHEAD < n_i:
                        issue_preb(i + LOOKAHEAD)
                    pre = nc.gpsimd.memset(bt[:, 0, BROW:], 0.0)
                    if groups:
                        groups[-1][2] = pre  # pre also carries prev updates
                        _add_dep(_mi(pre), _mi(groups[-1][1][-1]), sync=False,
                                 reason="chain")
                    sc = nc.gpsimd.local_scatter(
                        out_ap=bt[:, 0, 0:BROW],
                        data_ap=md[:, i, :],
                        idxs_ap=bix[:],
                        channels=128,
                        num_elems=BROW,
                        num_idxs=NSL,
                    )
                    _add_dep(_mi(sc), _mi(pre), sync=False, reason="chain")
                    groups.append([pre, [sc], None])

                    for ch in range(2):
                        pm = mmp.tile([128, 2, SW], F32, tag="mm")
                        for di in range(K):
                            nc.tensor.matmul(
                                pm[:],
                                ft[:, i + di, ch * 128:(ch + 1) * 128],
                                bt[:, :, di * SW:(di + 1) * SW],
                                start=(di == 0),
                                stop=(di == K - 1),
                            )
                        if ch == 0:
                            nc.scalar.copy(
                                out=orow[:, 2 * ii:2 * ii + 2, ch, :],
                                in_=pm[:],
                            )
                        else:
                            nc.vector.tensor_copy(
                                orow[:, 2 * ii:2 * ii + 2, ch, :], pm[:],
                            )
                if ib0 + IB >= n_i:
                    # final block: per-row-pair DMAs to shorten the tail
                    for ii in range(IB):
                        for ch in range(2):
                            dma_eng = nc.scalar if ch == 0 else nc.sync
                            dma_eng.dma_start(
                                out=out[ch * 128:(ch + 1) * 128,
                                        2 * (ib0 + ii):2 * (ib0 + ii) + 2, :],
                                in_=orow[:, 2 * ii:2 * ii + 2, ch, :],
                            )
                else:
                    for ch in range(2):
                        dma_eng = nc.scalar if ch == 0 else nc.sync
                        dma_eng.dma_start(
                            out=out[ch * 128:(ch + 1) * 128,
                                    2 * ib0:2 * ib0 + yb, :],
                            in_=orow[:, :, ch, :],
                        )
            term = nc.gpsimd.memset(bt[:, 1, BROW:], 0.0)
            _add_dep(_mi(term), _mi(groups[-1][1][-1]), sync=False,
                     reason="chain")
            groups[-1][2] = term

    if relocate:
        for pre, scats, post in groups:
            relocate_sync([pre], scats, [post])
        split_sync(nc)
    return nc


def finalize_for_hw(nc):
    assert mybir.codegen_inst_isa_subclasses(nc)
    return nc


_PROGRAM = None


def _get_program():
    global _PROGRAM
    if _PROGRAM is None:
        _PROGRAM = finalize_for_hw(build_program())
    return _PROGRAM


def kernel(features: np.ndarray, masks: np.ndarray) -> np.ndarray:
    from concourse.bass_utils import run_bass_kernel_spmd

    features = np.ascontiguousarray(features, dtype=np.float32)
    masks = np.ascontiguousarray(masks, dtype=np.float32)
    fpad = np.pad(features, ((0, 0), (0, 0), (PAD, PAD), (0, 0)))
    bix = host_bidx()

    in_maps = []
    for core in range(N_CORES):
        n, q = divmod(core, 4)
        ftt = fpad[n, :, QH * q:QH * q + R_IN, :].transpose(2, 1, 0)
        mshard = masks[n, :, 2 * N_I * q:2 * N_I * (q + 1), :]
        in_maps.append({
            "featt": np.ascontiguousarray(ftt.astype(np.float16)),
            "maskq": host_maskq(mshard),
            "preb": host_prebanded(mshard),
            "bidx": bix,
        })

    nc = _get_program()
    trace = os.environ.get("CARAFE_TRACE") == "1"
    res = run_bass_kernel_spmd(nc, in_maps, list(range(N_CORES)), trace=trace)
    kernel.last_results = res

    out = np.empty((N, C, SH, SW), dtype=np.float32)
    for core in range(N_CORES):
        n, q = divmod(core, 4)
        out[n, :, 2 * N_I * q:2 * N_I * (q + 1), :] = (
            res.results[core]["out"].astype(np.float32))
    return out


# revision 21
# speedup vs baseline: 4.2101x; 1.0271x over previous
"""CARAFE D4: fp16 banded-matmul with split B-tile construction.

out[c, y, x] = sum_di sum_dj fpad[c, y//2+di, x//2+dj] * m[di*5+dj, y, x]

For a fixed input row index i (covering output rows y=2i and 2i+1, which use
the same feature rows) and tap row di, the contribution over all (yp, x) is a
matmul contracting over the padded input column j' (128 lanes):

    out_i[c, (yp, x)] += sum_{j'} ftT[j', r=i+di, c] * B_di[j', (yp, x)]

where B_di[j', yp, x] = m[(di, dj), 2i+yp, x] at dj = j' - x//2 + 2 (banded,
5 diagonals per yp, zeros elsewhere).  The 5 di-taps accumulate in a full
512-wide PSUM bank.  All matmul operands are fp16 (1 cycle/row on PE vs 4
for fp32; PSUM accumulation stays fp32).

B tiles are built two ways, balancing GPSIMD against DMA bandwidth:
 - the yp=0 slabs by one GPSIMD local_scatter per row pair of fp16 mask
   values from a host-pregathered tensor with a static index table (the
   banded slot positions are y-independent), and
 - all five yp=1 slabs by a plain dense DMA of host-prebanded rows
   (zeros included) straight into the B tile, software-pipelined several
   row-pairs ahead of the compute wave.

The local_scatter extended-ISA instruction cannot carry semaphore ops through
this walrus build, so its sync is relocated onto adjacent Pool-engine memsets
(sound: Q7 execution is strict FIFO per engine), and a final pass splits any
instruction with more than one wait into standalone sequencer NOPs.
"""

import os

import numpy as np

import concourse.bass as bass
import concourse.mybir as mybir
import concourse.tile as tile
from concourse import library_config

F32 = mybir.dt.float32
F16 = mybir.dt.float16
I16 = mybir.dt.int16
_add_dep = bass._add_dep_helper

N, C, H, W = 2, 256, 128, 128
K = 5
S = 2
PAD = K // 2
SH, SW = H * S, W * S

N_CORES = 8
QH = H // 4          # 32 input rows per core
R_IN = QH + 2 * PAD  # 36 padded feature rows per core
N_I = QH             # 32 output row-pairs per core
YB = 4               # y rows per output DMA batch (2 i's)
NSL = K * K * 2      # 50 scatter slots (di, dj, px) for the yp=0 slabs
KDMA = K             # all yp=1 slabs arrive by dense prebanded DMA
FTCH = 12            # feature rows per load chunk


def _mi(x):
    return getattr(x, "ins", x)


def relocate_sync(pres, scats, posts):
    """Move the scatters' semaphore waits onto `pres` and updates onto
    `posts` (all chained in Pool-engine program order via nosync deps; Q7
    execution is strict FIFO per engine, so advancing waits and delaying
    updates across the group is sync-preserving).  Waits merge by max per
    semaphore, updates merge by sum."""
    def si_of(inst):
        si = inst.sync_info
        if si is None:
            return [], []
        return list(si.on_wait or []), list(si.on_update or [])

    wmax, uacc = {}, {}
    for s in scats:
        w, u = si_of(_mi(s))
        for x in w:
            assert x.sync_type == "semaphore" and x.wait_mode == "sem-ge-imm", x
            prev = wmax.get(x.id)
            if prev is None or x.wait_value > prev.wait_value:
                wmax[x.id] = x
        for x in u:
            assert x.sync_type == "semaphore" and x.update_mode in (
                "sem-inc", "sem-add-imm"), x
            prev = uacc.get(x.id)
            if prev is None:
                uacc[x.id] = mybir.SyncUpdate(
                    sync_type="semaphore", id=x.id, ant_name=x.ant_name,
                    update_mode="sem-add-imm", update_value=x.update_value)
            else:
                prev.update_value = prev.update_value + x.update_value
        _mi(s).sync_info = mybir.SyncInfo(on_wait=[], on_update=[])

    for carrier in pres:
        ci = _mi(carrier)
        cw, cu = si_of(ci)
        for w in cw:
            inc = wmax.pop(w.id, None)
            if inc is not None and inc.wait_value > w.wait_value:
                w.wait_value = inc.wait_value
        take = list(wmax.values())
        wmax.clear()
        ci.sync_info = mybir.SyncInfo(on_wait=cw + take, on_update=cu)
        break
    assert not wmax

    for carrier in posts:
        ci = _mi(carrier)
        cw, cu = si_of(ci)
        for u in cu:
            inc = uacc.pop(u.id, None)
            if inc is not None:
                u.update_value = u.update_value + inc.update_value
                u.update_mode = "sem-add-imm"
        take = list(uacc.values())
        uacc.clear()
        ci.sync_info = mybir.SyncInfo(on_wait=cw, on_update=cu + take)
        break
    assert not uacc


def split_sync(nc):
    """Enforce <=1 wait and <=1 update per instruction (this walrus build's
    events capacity), hoisting excess waits onto standalone same-engine
    sequencer NOPs placed immediately before (sync-equivalent).  Also hoists
    a wait that shares its semaphore with the instruction's own update."""
    for f in nc.m.functions:
        for b in f.blocks:
            lst = b.instructions
            i = 0
            while i < len(lst):
                inst = lst[i]
                si = getattr(inst, "sync_info", None)
                if si is None:
                    i += 1
                    continue
                w = list(si.on_wait or [])
                u = list(si.on_update or [])
                assert len(u) <= 1, (inst.name, u)
                uids = {x.id for x in u}
                conflict = any(x.id in uids for x in w) or (
                    w and any(x.update_mode == "sem-add-imm" for x in u))
                if len(w) <= 1 and not conflict:
                    i += 1
                    continue
                if (w and w[-1].id not in uids
                        and not any(x.update_mode == "sem-add-imm" for x in u)):
                    move, keep = w[:-1], w[-1:]
                else:
                    move, keep = w, []
                for wt in move:
                    nop = mybir.InstNoOp(
                        name=f"{inst.name}-ss{i}", text_hint="syncsplit")
                    nop.engine = inst.engine
                    nop.sync_info = mybir.SyncInfo(on_wait=[wt], on_update=[])
                    nc.register_instruction(nop, overwrite=True)
                    lst.insert(i, nop)
                    i += 1
                inst.sync_info = mybir.SyncInfo(on_wait=keep, on_update=u)
                i += 1


def _gather_slots(m, yp, dis):
    """slots[j', i, (di in dis, dj, px)] = m[di*5+dj, 2i+yp, 2j'-2dj+4+px]
    (0 where x is out of bounds), fp16.  m: [K*K, 2*ni, SW] fp32."""
    kk, ny, sw = m.shape
    ni = ny // 2
    mr = m.reshape(K, K, ni, 2, sw)  # [di, dj, i, yp, x]
    d = np.zeros((128, ni, len(dis), K, 2), dtype=np.float16)
    for dj in range(K):
        for px in range(2):
            x = 2 * np.arange(128) - 2 * dj + 4 + px  # [128]
            valid = (x >= 0) & (x < sw)
            xc = np.clip(x, 0, sw - 1)
            sel = mr[dis, dj][:, :, yp][:, :, xc]     # [dis, i, 128]
            sel = sel * valid[None, None, :]
            d[:, :, :, dj, px] = sel.transpose(2, 1, 0).astype(np.float16)
    return d.reshape(128, ni, len(dis) * K * 2)


def host_maskq(mask_shard: np.ndarray):
    """Scatter payloads for the yp=0 slabs (50 slots)."""
    return np.ascontiguousarray(_gather_slots(mask_shard, 0, list(range(K))))


def host_banded(mask_shard: np.ndarray, yp: int, i0: int, i1: int):
    """Dense banded slabs pb[j', i-i0, di, x] = m[di*5+dj, 2i+yp, x] at
    x = 2j'-2dj+4+px, else 0 (i in [i0, i1))."""
    kk, ny, sw = mask_shard.shape
    ni = ny // 2
    mr = mask_shard.reshape(K, K, ni, 2, sw)
    pb = np.zeros((128, i1 - i0, K, SW), dtype=np.float16)
    j = np.arange(128)
    for dj in range(K):
        for px in range(2):
            x = 2 * j - 2 * dj + 4 + px
            valid = (x >= 0) & (x < sw)
            jv = j[valid]
            pb[jv, :, :, x[valid]] = (
                mr[:, dj, i0:i1, yp][:, :, x[valid]]
                .transpose(2, 1, 0).astype(np.float16)
            )
    return pb


def host_prebanded(mask_shard: np.ndarray):
    return np.ascontiguousarray(host_banded(mask_shard, 1, 0, N_I))


def host_bidx():
    """Static scatter index table: slot (di, dj, px) of partition j' goes
    to position di*SW + 2j' - 2dj + 4 + px (-1 where x OOB)."""
    idx = np.full((128, NSL), -1, dtype=np.int16)
    for j in range(128):
        s = 0
        for di in range(K):
            for dj in range(K):
                for px in range(2):
                    x = 2 * j - 2 * dj + 4 + px
                    if 0 <= x < SW:
                        idx[j, s] = di * SW + x
                    s += 1
    return np.ascontiguousarray(idx)


def build_program(n_i: int = N_I, r_in: int = R_IN, relocate: bool = True,
                  detect_races: bool = False, yb: int = YB, lookahead: int = 6,
                  bt_bufs: int = 6, orow_bufs: int = 3, mm_bufs: int = 6):
    nc = bass.Bass(detect_race_conditions=detect_races)

    featt = nc.dram_tensor("featt", [128, r_in, C], F16, kind="ExternalInput")
    maskq = nc.dram_tensor(
        "maskq", [128, n_i, NSL], F16, kind="ExternalInput"
    )
    preb = nc.dram_tensor(
        "preb", [128, n_i, KDMA, SW], F16, kind="ExternalInput"
    )
    bidx = nc.dram_tensor("bidx", [128, NSL], I16, kind="ExternalInput")
    out = nc.dram_tensor("out", [C, 2 * n_i, SW], F16, kind="ExternalOutput")

    groups = []
    BROW = K * SW  # 1280 elements per yp

    with tile.TileContext(nc) as tc:
        with (
            tc.tile_pool(name="const", bufs=1) as constp,
            tc.tile_pool(name="ft", bufs=1) as ftp,
            tc.tile_pool(name="maskq", bufs=1) as mdp,
            tc.tile_pool(name="btile", bufs=bt_bufs) as bp,
            tc.tile_pool(name="orow", bufs=orow_bufs) as orowp,
            tc.tile_pool(name="mm", bufs=mm_bufs, space="PSUM") as mmp,
        ):
            nc.gpsimd.load_library(library_config.local_scatter)
            bix = constp.tile([128, NSL], I16, tag="bix")
            nc.scalar.dma_start(out=bix[:], in_=bidx[:])

            # B tiles are allocated LOOKAHEAD iterations early so their
            # yp=1 prebanded DMA can be issued ahead of the compute wave.
            LOOKAHEAD = lookahead
            bts = {}

            def issue_preb(i, eng=None):
                bt = bp.tile([128, 2, K * SW + 2], F16, tag="bt")
                if eng is None:
                    eng = nc.scalar if i % 2 == 0 else nc.sync
                eng.dma_start(out=bt[:, 1, 0:KDMA * SW], in_=preb[:, i])
                bts[i] = bt

            # scatter payloads resident: [j', i, slots] fp16, loaded in
            # chunks so the first scatters can start early
            md = mdp.tile([128, n_i, NSL], F16)
            ft = ftp.tile([128, r_in, C], F16)
            nc.sync.dma_start(out=md[:, 0:4], in_=maskq[:, 0:4])
            nc.sync.dma_start(out=ft[:, 0:6, :], in_=featt[:, 0:6, :])
            issue_preb(0)
            issue_preb(1)
            nc.sync.dma_start(out=md[:, 4:8], in_=maskq[:, 4:8])
            issue_preb(2)
            nc.sync.dma_start(out=ft[:, 6:12, :], in_=featt[:, 6:12, :])
            issue_preb(3)
            nc.sync.dma_start(out=md[:, 8:12], in_=maskq[:, 8:12])
            for i in range(4, LOOKAHEAD):
                issue_preb(i)
            nc.sync.dma_start(out=md[:, 12:], in_=maskq[:, 12:])
            for r0 in range(12, r_in, FTCH):
                r1 = min(r0 + FTCH, r_in)
                nc.sync.dma_start(
                    out=ft[:, r0:r1, :], in_=featt[:, r0:r1, :]
                )

            # ---- main loop over output row pairs ----
            IB = yb // 2
            for ib0 in range(0, n_i, IB):
                orow = orowp.tile([128, yb, 2, SW], F16, tag="orow")
                for ii in range(IB):
                    i = ib0 + ii
                    bt = bts.pop(i)
                    if i + LOOKAHEAD < n_i:
                        issue_preb(i + LOOKAHEAD)
                    pre = nc.gpsimd.memset(bt[:, 0, BROW:], 0.0)
                    if groups:
                        groups[-1][2] = pre  # pre also carries prev updates
                        _add_dep(_mi(pre), _mi(groups[-1][1][-1]), sync=False,
                                 reason="chain")
                    sc = nc.gpsimd.local_scatter(
                        out_ap=bt[:, 0, 0:BROW],
                        data_ap=md[:, i, :],
                        idxs_ap=bix[:],
                        channels=128,
                        num_elems=BROW,
                        num_idxs=NSL,
                    )
                    _add_dep(_mi(sc), _mi(pre), sync=False, reason="chain")
                    groups.append([pre, [sc], None])

                    for ch in range(2):
                        pm = mmp.tile([128, 2, SW], F32, tag="mm")
                        for di in range(K):
                            nc.tensor.matmul(
                                pm[:],
                                ft[:, i + di, ch * 128:(ch + 1) * 128],
                                bt[:, :, di * SW:(di + 1) * SW],
                                start=(di == 0),
                                stop=(di == K - 1),
                            )
                        if ch == 0:
                            nc.scalar.copy(
                                out=orow[:, 2 * ii:2 * ii + 2, ch, :],
                                in_=pm[:],
                            )
                        else:
                            nc.vector.tensor_copy(
                                orow[:, 2 * ii:2 * ii + 2, ch, :], pm[:],
                            )
                if ib0 + IB >= n_i:
                    # final block: per-row-pair DMAs to shorten the tail
                    for ii in range(IB):
                        for ch in range(2):
                            dma_eng = nc.scalar if ch == 0 else nc.sync
                            dma_eng.dma_start(
                                out=out[ch * 128:(ch + 1) * 128,
                                        2 * (ib0 + ii):2 * (ib0 + ii) + 2, :],
                                in_=orow[:, 2 * ii:2 * ii + 2, ch, :],
                            )
                else:
                    for ch in range(2):
                        dma_eng = nc.scalar if ch == 0 else nc.sync
                        dma_eng.dma_start(
                            out=out[ch * 128:(ch + 1) * 128,
                                    2 * ib0:2 * ib0 + yb, :],
                            in_=orow[:, :, ch, :],
                        )
            term = nc.gpsimd.memset(bt[:, 1, BROW:], 0.0)
            _add_dep(_mi(term), _mi(groups[-1][1][-1]), sync=False,
                     reason="chain")
            groups[-1][2] = term

    if relocate:
        for pre, scats, post in groups:
            relocate_sync([pre], scats, [post])
        split_sync(nc)
    return nc


def finalize_for_hw(nc):
    assert mybir.codegen_inst_isa_subclasses(nc)
    return nc


_PROGRAM = None


def _get_program():
    global _PROGRAM
    if _PROGRAM is None:
        _PROGRAM = finalize_for_hw(build_program())
    return _PROGRAM


def kernel(features: np.ndarray, masks: np.ndarray) -> np.ndarray:
    from concourse.bass_utils import run_bass_kernel_spmd

    features = np.ascontiguousarray(features, dtype=np.float32)
    masks = np.ascontiguousarray(masks, dtype=np.float32)
    fpad = np.pad(features, ((0, 0), (0, 0), (PAD, PAD), (0, 0)))
    bix = host_bidx()

    in_maps = []
    for core in range(N_CORES):
        n, q = divmod(core, 4)
        ftt = fpad[n, :, QH * q:QH * q + R_IN, :].transpose(2, 1, 0)
        mshard = masks[n, :, 2 * N_I * q:2 * N_I * (q + 1), :]
        in_maps.append({
            "featt": np.ascontiguousarray(ftt.astype(np.float16)),
            "maskq": host_maskq(mshard),
            "preb": host_prebanded(mshard),
            "bidx": bix,
        })

    nc = _get_program()
    trace = os.environ.get("CARAFE_TRACE") == "1"
    res = run_bass_kernel_spmd(nc, in_maps, list(range(N_CORES)), trace=trace)
    kernel.last_results = res

    out = np.empty((N, C, SH, SW), dtype=np.float32)
    for core in range(N_CORES):
        n, q = divmod(core, 4)
        out[n, :, 2 * N_I * q:2 * N_I * (q + 1), :] = (
            res.results[core]["out"].astype(np.float32))
    return out


# revision 22
# speedup vs baseline: 4.2207x; 1.0025x over previous
"""CARAFE D4: fp16 banded-matmul with split B-tile construction.

out[c, y, x] = sum_di sum_dj fpad[c, y//2+di, x//2+dj] * m[di*5+dj, y, x]

For a fixed input row index i (covering output rows y=2i and 2i+1, which use
the same feature rows) and tap row di, the contribution over all (yp, x) is a
matmul contracting over the padded input column j' (128 lanes):

    out_i[c, (yp, x)] += sum_{j'} ftT[j', r=i+di, c] * B_di[j', (yp, x)]

where B_di[j', yp, x] = m[(di, dj), 2i+yp, x] at dj = j' - x//2 + 2 (banded,
5 diagonals per yp, zeros elsewhere).  The 5 di-taps accumulate in a full
512-wide PSUM bank.  All matmul operands are fp16 (1 cycle/row on PE vs 4
for fp32; PSUM accumulation stays fp32).

B tiles are built two ways, balancing GPSIMD against DMA bandwidth:
 - the yp=0 slabs by one GPSIMD local_scatter per row pair of fp16 mask
   values from a host-pregathered tensor with a static index table (the
   banded slot positions are y-independent), and
 - all five yp=1 slabs by a plain dense DMA of host-prebanded rows
   (zeros included) straight into the B tile, software-pipelined several
   row-pairs ahead of the compute wave.

The local_scatter extended-ISA instruction cannot carry semaphore ops through
this walrus build, so its sync is relocated onto adjacent Pool-engine memsets
(sound: Q7 execution is strict FIFO per engine), and a final pass splits any
instruction with more than one wait into standalone sequencer NOPs.
"""

import os

import numpy as np

import concourse.bass as bass
import concourse.mybir as mybir
import concourse.tile as tile
from concourse import library_config

F32 = mybir.dt.float32
F16 = mybir.dt.float16
I16 = mybir.dt.int16
_add_dep = bass._add_dep_helper

N, C, H, W = 2, 256, 128, 128
K = 5
S = 2
PAD = K // 2
SH, SW = H * S, W * S

N_CORES = 8
QH = H // 4          # 32 input rows per core
R_IN = QH + 2 * PAD  # 36 padded feature rows per core
N_I = QH             # 32 output row-pairs per core
YB = 8               # y rows per output DMA batch (4 i's)
NSL = K * K * 2      # 50 scatter slots (di, dj, px) for the yp=0 slabs
KDMA = K             # all yp=1 slabs arrive by dense prebanded DMA
FTCH = 12            # feature rows per load chunk


def _mi(x):
    return getattr(x, "ins", x)


def relocate_sync(pres, scats, posts):
    """Move the scatters' semaphore waits onto `pres` and updates onto
    `posts` (all chained in Pool-engine program order via nosync deps; Q7
    execution is strict FIFO per engine, so advancing waits and delaying
    updates across the group is sync-preserving).  Waits merge by max per
    semaphore, updates merge by sum."""
    def si_of(inst):
        si = inst.sync_info
        if si is None:
            return [], []
        return list(si.on_wait or []), list(si.on_update or [])

    wmax, uacc = {}, {}
    for s in scats:
        w, u = si_of(_mi(s))
        for x in w:
            assert x.sync_type == "semaphore" and x.wait_mode == "sem-ge-imm", x
            prev = wmax.get(x.id)
            if prev is None or x.wait_value > prev.wait_value:
                wmax[x.id] = x
        for x in u:
            assert x.sync_type == "semaphore" and x.update_mode in (
                "sem-inc", "sem-add-imm"), x
            prev = uacc.get(x.id)
            if prev is None:
                uacc[x.id] = mybir.SyncUpdate(
                    sync_type="semaphore", id=x.id, ant_name=x.ant_name,
                    update_mode="sem-add-imm", update_value=x.update_value)
            else:
                prev.update_value = prev.update_value + x.update_value
        _mi(s).sync_info = mybir.SyncInfo(on_wait=[], on_update=[])

    for carrier in pres:
        ci = _mi(carrier)
        cw, cu = si_of(ci)
        for w in cw:
            inc = wmax.pop(w.id, None)
            if inc is not None and inc.wait_value > w.wait_value:
                w.wait_value = inc.wait_value
        take = list(wmax.values())
        wmax.clear()
        ci.sync_info = mybir.SyncInfo(on_wait=cw + take, on_update=cu)
        break
    assert not wmax

    for carrier in posts:
        ci = _mi(carrier)
        cw, cu = si_of(ci)
        for u in cu:
            inc = uacc.pop(u.id, None)
            if inc is not None:
                u.update_value = u.update_value + inc.update_value
                u.update_mode = "sem-add-imm"
        take = list(uacc.values())
        uacc.clear()
        ci.sync_info = mybir.SyncInfo(on_wait=cw, on_update=cu + take)
        break
    assert not uacc


def split_sync(nc):
    """Enforce <=1 wait and <=1 update per instruction (this walrus build's
    events capacity), hoisting excess waits onto standalone same-engine
    sequencer NOPs placed immediately before (sync-equivalent).  Also hoists
    a wait that shares its semaphore with the instruction's own update."""
    for f in nc.m.functions:
        for b in f.blocks:
            lst = b.instructions
            i = 0
            while i < len(lst):
                inst = lst[i]
                si = getattr(inst, "sync_info", None)
                if si is None:
                    i += 1
                    continue
                w = list(si.on_wait or [])
                u = list(si.on_update or [])
                assert len(u) <= 1, (inst.name, u)
                uids = {x.id for x in u}
                conflict = any(x.id in uids for x in w) or (
                    w and any(x.update_mode == "sem-add-imm" for x in u))
                if len(w) <= 1 and not conflict:
                    i += 1
                    continue
                if (w and w[-1].id not in uids
                        and not any(x.update_mode == "sem-add-imm" for x in u)):
                    move, keep = w[:-1], w[-1:]
                else:
                    move, keep = w, []
                for wt in move:
                    nop = mybir.InstNoOp(
                        name=f"{inst.name}-ss{i}", text_hint="syncsplit")
                    nop.engine = inst.engine
                    nop.sync_info = mybir.SyncInfo(on_wait=[wt], on_update=[])
                    nc.register_instruction(nop, overwrite=True)
                    lst.insert(i, nop)
                    i += 1
                inst.sync_info = mybir.SyncInfo(on_wait=keep, on_update=u)
                i += 1


def _gather_slots(m, yp, dis):
    """slots[j', i, (di in dis, dj, px)] = m[di*5+dj, 2i+yp, 2j'-2dj+4+px]
    (0 where x is out of bounds), fp16.  m: [K*K, 2*ni, SW] fp32."""
    kk, ny, sw = m.shape
    ni = ny // 2
    mr = m.reshape(K, K, ni, 2, sw)  # [di, dj, i, yp, x]
    d = np.zeros((128, ni, len(dis), K, 2), dtype=np.float16)
    for dj in range(K):
        for px in range(2):
            x = 2 * np.arange(128) - 2 * dj + 4 + px  # [128]
            valid = (x >= 0) & (x < sw)
            xc = np.clip(x, 0, sw - 1)
            sel = mr[dis, dj][:, :, yp][:, :, xc]     # [dis, i, 128]
            sel = sel * valid[None, None, :]
            d[:, :, :, dj, px] = sel.transpose(2, 1, 0).astype(np.float16)
    return d.reshape(128, ni, len(dis) * K * 2)


def host_maskq(mask_shard: np.ndarray):
    """Scatter payloads for the yp=0 slabs (50 slots)."""
    return np.ascontiguousarray(_gather_slots(mask_shard, 0, list(range(K))))


def host_banded(mask_shard: np.ndarray, yp: int, i0: int, i1: int):
    """Dense banded slabs pb[j', i-i0, di, x] = m[di*5+dj, 2i+yp, x] at
    x = 2j'-2dj+4+px, else 0 (i in [i0, i1))."""
    kk, ny, sw = mask_shard.shape
    ni = ny // 2
    mr = mask_shard.reshape(K, K, ni, 2, sw)
    pb = np.zeros((128, i1 - i0, K, SW), dtype=np.float16)
    j = np.arange(128)
    for dj in range(K):
        for px in range(2):
            x = 2 * j - 2 * dj + 4 + px
            valid = (x >= 0) & (x < sw)
            jv = j[valid]
            pb[jv, :, :, x[valid]] = (
                mr[:, dj, i0:i1, yp][:, :, x[valid]]
                .transpose(2, 1, 0).astype(np.float16)
            )
    return pb


def host_prebanded(mask_shard: np.ndarray):
    return np.ascontiguousarray(host_banded(mask_shard, 1, 0, N_I))


def host_bidx():
    """Static scatter index table: slot (di, dj, px) of partition j' goes
    to position di*SW + 2j' - 2dj + 4 + px (-1 where x OOB)."""
    idx = np.full((128, NSL), -1, dtype=np.int16)
    for j in range(128):
        s = 0
        for di in range(K):
            for dj in range(K):
                for px in range(2):
                    x = 2 * j - 2 * dj + 4 + px
                    if 0 <= x < SW:
                        idx[j, s] = di * SW + x
                    s += 1
    return np.ascontiguousarray(idx)


def build_program(n_i: int = N_I, r_in: int = R_IN, relocate: bool = True,
                  detect_races: bool = False, yb: int = YB, lookahead: int = 6,
                  bt_bufs: int = 6, orow_bufs: int = 3, mm_bufs: int = 6):
    nc = bass.Bass(detect_race_conditions=detect_races)

    featt = nc.dram_tensor("featt", [128, r_in, C], F16, kind="ExternalInput")
    maskq = nc.dram_tensor(
        "maskq", [128, n_i, NSL], F16, kind="ExternalInput"
    )
    preb = nc.dram_tensor(
        "preb", [128, n_i, KDMA, SW], F16, kind="ExternalInput"
    )
    bidx = nc.dram_tensor("bidx", [128, NSL], I16, kind="ExternalInput")
    out = nc.dram_tensor("out", [C, 2 * n_i, SW], F16, kind="ExternalOutput")

    groups = []
    BROW = K * SW  # 1280 elements per yp

    with tile.TileContext(nc) as tc:
        with (
            tc.tile_pool(name="const", bufs=1) as constp,
            tc.tile_pool(name="ft", bufs=1) as ftp,
            tc.tile_pool(name="maskq", bufs=1) as mdp,
            tc.tile_pool(name="btile", bufs=bt_bufs) as bp,
            tc.tile_pool(name="orow", bufs=orow_bufs) as orowp,
            tc.tile_pool(name="mm", bufs=mm_bufs, space="PSUM") as mmp,
        ):
            nc.gpsimd.load_library(library_config.local_scatter)
            bix = constp.tile([128, NSL], I16, tag="bix")
            nc.scalar.dma_start(out=bix[:], in_=bidx[:])

            # B tiles are allocated LOOKAHEAD iterations early so their
            # yp=1 prebanded DMA can be issued ahead of the compute wave.
            LOOKAHEAD = lookahead
            bts = {}

            def issue_preb(i, eng=None):
                bt = bp.tile([128, 2, K * SW + 2], F16, tag="bt")
                if eng is None:
                    eng = nc.scalar if i % 2 == 0 else nc.sync
                eng.dma_start(out=bt[:, 1, 0:KDMA * SW], in_=preb[:, i])
                bts[i] = bt

            # scatter payloads resident: [j', i, slots] fp16, loaded in
            # chunks so the first scatters can start early
            md = mdp.tile([128, n_i, NSL], F16)
            ft = ftp.tile([128, r_in, C], F16)
            nc.sync.dma_start(out=md[:, 0:4], in_=maskq[:, 0:4])
            nc.sync.dma_start(out=ft[:, 0:6, :], in_=featt[:, 0:6, :])
            issue_preb(0)
            issue_preb(1)
            nc.sync.dma_start(out=md[:, 4:8], in_=maskq[:, 4:8])
            issue_preb(2)
            nc.sync.dma_start(out=ft[:, 6:12, :], in_=featt[:, 6:12, :])
            issue_preb(3)
            nc.sync.dma_start(out=md[:, 8:12], in_=maskq[:, 8:12])
            for i in range(4, LOOKAHEAD):
                issue_preb(i)
            nc.sync.dma_start(out=md[:, 12:], in_=maskq[:, 12:])
            for r0 in range(12, r_in, FTCH):
                r1 = min(r0 + FTCH, r_in)
                nc.sync.dma_start(
                    out=ft[:, r0:r1, :], in_=featt[:, r0:r1, :]
                )

            # ---- main loop over output row pairs ----
            IB = yb // 2
            for ib0 in range(0, n_i, IB):
                orow = orowp.tile([128, yb, 2, SW], F16, tag="orow")
                for ii in range(IB):
                    i = ib0 + ii
                    bt = bts.pop(i)
                    if i + LOOKAHEAD < n_i:
                        issue_preb(i + LOOKAHEAD)
                    pre = nc.gpsimd.memset(bt[:, 0, BROW:], 0.0)
                    if groups:
                        groups[-1][2] = pre  # pre also carries prev updates
                        _add_dep(_mi(pre), _mi(groups[-1][1][-1]), sync=False,
                                 reason="chain")
                    sc = nc.gpsimd.local_scatter(
                        out_ap=bt[:, 0, 0:BROW],
                        data_ap=md[:, i, :],
                        idxs_ap=bix[:],
                        channels=128,
                        num_elems=BROW,
                        num_idxs=NSL,
                    )
                    _add_dep(_mi(sc), _mi(pre), sync=False, reason="chain")
                    groups.append([pre, [sc], None])

                    for ch in range(2):
                        pm = mmp.tile([128, 2, SW], F32, tag="mm")
                        for di in range(K):
                            nc.tensor.matmul(
                                pm[:],
                                ft[:, i + di, ch * 128:(ch + 1) * 128],
                                bt[:, :, di * SW:(di + 1) * SW],
                                start=(di == 0),
                                stop=(di == K - 1),
                            )
                        if ch == 0:
                            nc.scalar.copy(
                                out=orow[:, 2 * ii:2 * ii + 2, ch, :],
                                in_=pm[:],
                            )
                        else:
                            nc.vector.tensor_copy(
                                orow[:, 2 * ii:2 * ii + 2, ch, :], pm[:],
                            )
                if ib0 + IB >= n_i:
                    # final block: per-row-pair DMAs to shorten the tail
                    for ii in range(IB):
                        for ch in range(2):
                            dma_eng = nc.scalar if ch == 0 else nc.sync
                            dma_eng.dma_start(
                                out=out[ch * 128:(ch + 1) * 128,
                                        2 * (ib0 + ii):2 * (ib0 + ii) + 2, :],
                                in_=orow[:, 2 * ii:2 * ii + 2, ch, :],
                            )
                else:
                    for ch in range(2):
                        dma_eng = nc.scalar if ch == 0 else nc.sync
                        dma_eng.dma_start(
                            out=out[ch * 128:(ch + 1) * 128,
                                    2 * ib0:2 * ib0 + yb, :],
                            in_=orow[:, :, ch, :],
                        )
            term = nc.gpsimd.memset(bt[:, 1, BROW:], 0.0)
            _add_dep(_mi(term), _mi(groups[-1][1][-1]), sync=False,
                     reason="chain")
            groups[-1][2] = term

    if relocate:
        for pre, scats, post in groups:
            relocate_sync([pre], scats, [post])
        split_sync(nc)
    return nc


def finalize_for_hw(nc):
    assert mybir.codegen_inst_isa_subclasses(nc)
    return nc


_PROGRAM = None


def _get_program():
    global _PROGRAM
    if _PROGRAM is None:
        _PROGRAM = finalize_for_hw(build_program())
    return _PROGRAM


def kernel(features: np.ndarray, masks: np.ndarray) -> np.ndarray:
    from concourse.bass_utils import run_bass_kernel_spmd

    features = np.ascontiguousarray(features, dtype=np.float32)
    masks = np.ascontiguousarray(masks, dtype=np.float32)
    fpad = np.pad(features, ((0, 0), (0, 0), (PAD, PAD), (0, 0)))
    bix = host_bidx()

    in_maps = []
    for core in range(N_CORES):
        n, q = divmod(core, 4)
        ftt = fpad[n, :, QH * q:QH * q + R_IN, :].transpose(2, 1, 0)
        mshard = masks[n, :, 2 * N_I * q:2 * N_I * (q + 1), :]
        in_maps.append({
            "featt": np.ascontiguousarray(ftt.astype(np.float16)),
            "maskq": host_maskq(mshard),
            "preb": host_prebanded(mshard),
            "bidx": bix,
        })

    nc = _get_program()
    trace = os.environ.get("CARAFE_TRACE") == "1"
    res = run_bass_kernel_spmd(nc, in_maps, list(range(N_CORES)), trace=trace)
    kernel.last_results = res

    out = np.empty((N, C, SH, SW), dtype=np.float32)
    for core in range(N_CORES):
        n, q = divmod(core, 4)
        out[n, :, 2 * N_I * q:2 * N_I * (q + 1), :] = (
            res.results[core]["out"].astype(np.float32))
    return out
